# revision 21
# baseline (speedup 1.0000x reference)
"""BiLSTM-CRF loss kernel for trn2, one core = 32 sequences (data parallel).

Structure (v2):
- embedding gather via dma_gather(transpose) -> x (128=E, ntok) bf16, t-major
- BiLSTM: all-sigmoid gates (tanh(x)=2*sigma(2x)-1 folded into g-gate weights);
  per window: one bf16 bias matmul ([4,128] gate-bias lhsT x [4,1024] one-hot
  rhs) + 4 xproj matmuls per dir; per step: 4 Whh matmuls + sigmoid + c/h ops
- tail: emissions ([96,512] psum, wot duplicated so rows 48:96 mirror 0:48),
  exp into a folded EMpair[96, *] layout: top = EM blocks 0..255 (alpha),
  bottom = EM blocks 511..255 stored reversed (G chain); numerator A via
  one-hot amr per chunk; (49,48) pair histogram on gpsimd
- CRF: single fused chain over s=1..255 (alpha on partitions 0:48, G on
  48:96, block-diag exp(trans) lhsT + eend rank-1 inject), 2 G-only steps,
  then dot via SBUF->SBUF partition copy
Output per core: (1, 8) f32: [0]=numerator partial sum, [1]=denominator.
loss = (sum_den - sum_num) / B   (host combines the 8 cores)
"""
import numpy as np
import ml_dtypes

import concourse.bacc as bacc
import concourse.bass as bass
import concourse.mybir as mybir
from concourse.tile import TileContext

BF16 = ml_dtypes.bfloat16
F32 = np.float32
AF = mybir.ActivationFunctionType
ALU = mybir.AluOpType
DT = mybir.dt

T = 48


# --------------------------------------------------------------------------
# host-side preparation
# --------------------------------------------------------------------------

def prep_params(inp):
    """Build replicated parameter arrays (numpy) from raw inputs."""
    p = {}
    p["emb"] = np.ascontiguousarray(inp["emb"]).astype(BF16)

    def mk(Wih, Whh, bih, bhh):
        def reorder(W):
            i, f, g, o = np.split(np.asarray(W, F32), 4, 0)
            return np.concatenate([i, f, o, 2.0 * g], 0)
        WihT = np.ascontiguousarray(reorder(Wih).T).astype(BF16)   # (128, 512)
        WhhT = np.ascontiguousarray(reorder(Whh).T).astype(ml_dtypes.float8_e4m3)
        b = np.asarray(bih, F32) + np.asarray(bhh, F32)
        bi, bf_, bg, bo = np.split(b, 4)
        bias4 = np.stack([bi, bf_, bo, 2.0 * bg]).astype(BF16)     # (4, 128)
        return WihT, WhhT, bias4

    p["wiht_f"], p["whht_f"], p["bias4_f"] = mk(inp["Wih_f"], inp["Whh_f"], inp["bih_f"], inp["bhh_f"])
    p["wiht_b"], p["whht_b"], p["bias4_b"] = mk(inp["Wih_b"], inp["Whh_b"], inp["bih_b"], inp["bhh_b"])
    oh = np.zeros((2, 512), F32)
    oh[0, 0:256] = 1.0
    oh[1, 256:512] = 1.0
    p["onehot2"] = oh.astype(BF16)
    Wout = np.asarray(inp["Wout"], F32)     # (48, 256)
    H = Wout.shape[1] // 2
    wf = np.ascontiguousarray(Wout[:, :H].T)       # (128, 48)
    wb = np.ascontiguousarray(Wout[:, H:].T)
    z16 = np.zeros((128, 16), F32)
    p["wot112_f"] = np.concatenate([wf, z16, wf], 1).astype(BF16)   # (128, 112)
    p["wot112_b"] = np.concatenate([wb, z16, wb], 1).astype(BF16)
    c0 = np.log(T)
    exb = (np.asarray(inp["bout"], F32) - c0).reshape(T, 1)
    p["exbias112"] = np.concatenate([exb, np.zeros((16, 1), F32), exb], 0).astype(F32)
    trans = np.asarray(inp["trans"], F32)
    et = np.exp(trans)
    bd = np.zeros((112, 112), F32)
    bd[0:48, 0:48] = et            # alpha lhsT
    bd[64:112, 64:112] = et.T      # G lhsT
    p["bd112"] = bd.astype(BF16)
    ee = np.zeros((1, 112), F32)
    ee[0, 64:112] = np.exp(np.asarray(inp["end_trans"], F32))
    p["eend112"] = ee.astype(BF16)
    p["estart"] = np.exp(np.asarray(inp["start_trans"], F32)).reshape(T, 1).astype(F32)
    return p


def prep_shard(words, tags, mask):
    """Per-core input arrays. words/tags/mask: (b, L)."""
    b, L = words.shape
    ntok = b * L
    w_tm = np.ascontiguousarray(words.T).reshape(-1)
    tags_tm = np.ascontiguousarray(tags.T).reshape(-1)
    m_tm = np.ascontiguousarray(mask.T).reshape(-1).astype(F32)

    d = {}
    gi = w_tm.astype(np.int16).reshape(ntok // 16, 16).T          # (16, ntok/16)
    d["gidx"] = np.ascontiguousarray(np.tile(gi, (8, 1))).astype(np.int16)
    # gold-tag one-hot (masked), fp8: exact 0/1
    oh = (tags_tm[None, :] == np.arange(T)[:, None]) & (m_tm[None, :] > 0)
    d["ohm_all"] = oh.astype(ml_dtypes.float8_e4m3)               # (48, ntok)
    m_pad = np.pad(m_tm, (0, b))
    d["lsrow"] = (m_tm - m_pad[b:]).astype(BF16).reshape(1, ntok)
    return d


def host_extra(tags, mask, trans, start_trans, end_trans, bout):
    """Gold-path transition score + log-T mask term, in float64 on host."""
    m = np.asarray(mask, np.float64)
    t = np.asarray(tags, np.int64)
    tr = np.asarray(trans, np.float64)
    num = np.asarray(start_trans, np.float64)[t[:, 0]].sum()
    num += (tr[t[:, :-1], t[:, 1:]] * m[:, 1:]).sum()
    lengths = m.sum(1).astype(np.int64)
    last = t[np.arange(t.shape[0]), lengths - 1]
    num += np.asarray(end_trans, np.float64)[last].sum()
    num += (np.asarray(bout, np.float64)[t] * m).sum()
    den_extra = float(np.log(T)) * m.sum()
    return float(num), float(den_extra)


# --------------------------------------------------------------------------
# device kernel builder
# --------------------------------------------------------------------------

def build(L=512, BLOC=32, W=8, V=32000):
    ntok = L * BLOC
    NW = L // W
    half = L // 2           # 256
    NCH = ntok // 512       # emission chunks (32)
    NPCH = ntok // 128      # one-hot pchunks (128)
    REG = 32 * W            # 256: per-gate region in the window psum
    SBOT = (half + 2) * 32  # EMpair width: bottom blocks go up to s=257
    c0 = float(np.log(T))

    nc = bacc.Bacc()
    dp = nc.declare_dram_parameter
    g_gidx = dp("gidx", [128, ntok // 16], DT.int16, isOutput=False)
    g_ohm = dp("ohm_all", [T, ntok], DT.float8e4, isOutput=False)
    g_lsrow = dp("lsrow", [1, ntok], DT.bfloat16, isOutput=False)
    g_emb = dp("emb", [V, 128], DT.bfloat16, isOutput=False)
    g_w = {}
    for nm in ("wiht_f", "wiht_b"):
        g_w[nm] = dp(nm, [128, 512], DT.bfloat16, isOutput=False)
    for nm in ("whht_f", "whht_b"):
        g_w[nm] = dp(nm, [128, 512], DT.float8e4, isOutput=False)
    g_bias4 = {d: dp(f"bias4_{d}", [4, 128], DT.bfloat16, isOutput=False) for d in "fb"}
    g_onehot2 = dp("onehot2", [2, 512], DT.bfloat16, isOutput=False)
    g_wot112 = {d: dp(f"wot112_{d}", [128, 112], DT.bfloat16, isOutput=False) for d in "fb"}
    g_exbias112 = dp("exbias112", [112, 1], DT.float32, isOutput=False)
    g_bd112 = dp("bd112", [112, 112], DT.bfloat16, isOutput=False)
    g_eend112 = dp("eend112", [1, 112], DT.bfloat16, isOutput=False)
    g_estart = dp("estart", [T, 1], DT.float32, isOutput=False)
    g_out = dp("out", [1, 8], DT.float32, isOutput=True)

    with TileContext(nc) as tc:
        with tc.tile_pool(name="persist", bufs=1) as pp:
            # ---- persistent SBUF tiles
            Hf = pp.tile([128, ntok], DT.bfloat16, tag="Hf", name="Hf")
            Hb = pp.tile([128, ntok], DT.bfloat16, tag="Hb", name="Hb")
            wiht = {}
            whht = {}
            bias_lo = {}
            bias_hi = {}
            wot112 = {}
            for d in "fb":
                wiht[d] = pp.tile([128, 512], DT.bfloat16, tag=f"wiht{d}", name=f"wiht{d}")
                whht[d] = pp.tile([128, 512], DT.float8e4, tag=f"whht{d}", name=f"whht{d}")
                bias_lo[d] = pp.tile([2, 128], DT.bfloat16, tag=f"biaslo{d}", name=f"biaslo{d}")
                bias_hi[d] = pp.tile([2, 128], DT.bfloat16, tag=f"biashi{d}", name=f"biashi{d}")
                wot112[d] = pp.tile([128, 112], DT.bfloat16, tag=f"wot112{d}", name=f"wot112{d}")
            onehot2 = pp.tile([2, 512], DT.bfloat16, tag="onehot2", name="onehot2")
            exbias112 = pp.tile([112, 1], DT.float32, tag="exbias112", name="exbias112")
            bd112 = pp.tile([112, 112], DT.bfloat16, tag="bd112", name="bd112")
            eend112 = pp.tile([1, 112], DT.bfloat16, tag="eend112", name="eend112")
            estart = pp.tile([T, 1], DT.float32, tag="estart", name="estart")
            ohm_all = pp.tile([T, ntok], DT.float8e4, tag="ohmall", name="ohmall")
            lsrow = pp.tile([1, ntok], DT.bfloat16, tag="lsrow", name="lsrow")
            # small constants
            ones48col = pp.tile([T, 1], DT.float32, tag="ones48col", name="ones48col")
            # LSTM state
            cst = {d: pp.tile([128, BLOC], DT.bfloat16, tag=f"c{d}", name=f"c{d}") for d in "fb"}
            tmp1 = {d: pp.tile([128, BLOC], DT.bfloat16, tag=f"tmp1{d}", name=f"tmp1{d}") for d in "fb"}
            tmp2 = {d: pp.tile([128, BLOC], DT.bfloat16, tag=f"tmp2{d}", name=f"tmp2{d}") for d in "fb"}
            tct = {d: pp.tile([128, BLOC], DT.bfloat16, tag=f"tct{d}", name=f"tct{d}") for d in "fb"}
            jacc = {d: pp.tile([128, 1], DT.float32, tag=f"jacc{d}", name=f"jacc{d}") for d in "fb"}
            # numerator accumulators
            accA = pp.tile([T, NCH], DT.float32, tag="accA", name="accA")
            accA_red = pp.tile([T, 1], DT.float32, tag="accAred", name="accAred")
            junkA = pp.tile([T, 512], DT.bfloat16, tag="junkA", name="junkA")
            # CRF tiles
            EMpair = pp.tile([112, SBOT], DT.bfloat16, tag="EMpair", name="EMpair")
            ea = [pp.tile([112, BLOC], DT.bfloat16, tag=f"ea{i}", name=f"ea{i}") for i in range(2)]
            dcopy = pp.tile([112, BLOC], DT.bfloat16, tag="dcopy", name="dcopy")
            dott112 = pp.tile([112, BLOC], DT.float32, tag="dott112", name="dott112")
            dott_lo = pp.tile([T, BLOC], DT.float32, tag="dottlo", name="dottlo")
            logrow = pp.tile([1, BLOC], DT.float32, tag="logrow", name="logrow")
            dsum = pp.tile([1, 1], DT.float32, tag="dsum", name="dsum")
            tmp11 = pp.tile([1, 1], DT.float32, tag="tmp11", name="tmp11")
            out_sb = pp.tile([1, 8], DT.float32, tag="outsb", name="outsb")

            # ---- input DMAs
            S = nc.sync
            for d in "fb":
                S.dma_start(out=wiht[d][:], in_=g_w[f"wiht_{d}"][:])
                S.dma_start(out=whht[d][:], in_=g_w[f"whht_{d}"][:])
                S.dma_start(out=bias_lo[d][:], in_=g_bias4[d][0:2, :])
                S.dma_start(out=bias_hi[d][:], in_=g_bias4[d][2:4, :])
                S.dma_start(out=wot112[d][:], in_=g_wot112[d][:])
            S.dma_start(out=onehot2[:], in_=g_onehot2[:])
            S.dma_start(out=exbias112[:], in_=g_exbias112[:])
            S.dma_start(out=bd112[:], in_=g_bd112[:])
            S.dma_start(out=eend112[:], in_=g_eend112[:])
            S.dma_start(out=estart[:], in_=g_estart[:])
            S.dma_start(out=ohm_all[:], in_=g_ohm[:])
            S.dma_start(out=lsrow[:], in_=g_lsrow[:])

            # constants
            Vv = nc.vector
            Sc = nc.scalar
            Gp = nc.gpsimd
            Vv.memset(ones48col[:], 1.0)
            Vv.memset(accA[:], 0.0)
            Vv.memset(out_sb[:], 0.0)
            Vv.memset(ea[0][:], 0.0)
            Vv.memset(ea[1][:], 0.0)
            Vv.memset(EMpair[32:64, :], 0.0)
            for d in "fb":
                Vv.memset(cst[d][:], 0.0)

            # ---------------- LSTM ----------------
            Hdir = {"f": Hf, "b": Hb}
            with tc.tile_pool(name="lstm_ps", bufs=2, space="PSUM") as lpsp, \
                 tc.tile_pool(name="lstm_sb", bufs=3) as lsb, \
                 tc.tile_pool(name="xpool", bufs=1) as xp:
                x = xp.tile([128, ntok], DT.bfloat16, tag="x", name="x")
                gidx = xp.tile([128, ntok // 16], DT.int16, tag="gidx", name="gidx")
                S.dma_start(out=gidx[:], in_=g_gidx[:])
                GCH = min(ntok, 1024)
                _ng = ntok // GCH
                _order = []
                for _i in range((_ng + 1) // 2):
                    _order.append(_i)
                    if _ng - 1 - _i != _i:
                        _order.append(_ng - 1 - _i)
                for gc in _order:
                    nc.gpsimd.dma_gather(
                        out_ap=x[:, gc * GCH:(gc + 1) * GCH].rearrange(
                            "p (o n) -> p o n", o=1),
                        in_ap=g_emb[:],
                        idxs_ap=gidx[:, gc * (GCH // 16):(gc + 1) * (GCH // 16)],
                        num_idxs=GCH,
                        num_idxs_reg=GCH,
                        elem_size=128,
                        transpose=True,
                        single_packet=False,
                    )
                def window_mm_thunks(w, pf):
                    thunks = []
                    for d in "fb":
                        if d == "f":
                            x0 = w * W * BLOC
                        else:
                            x0 = (L - (w + 1) * W) * BLOC
                        # gate bias as one rank-2 bf16 matmul per psum bank
                        thunks.append(lambda d=d: nc.tensor.matmul(
                            pf[d][:, 0:512], bias_lo[d][:], onehot2[:],
                            start=True, stop=False, skip_group_check=True))
                        thunks.append(lambda d=d: nc.tensor.matmul(
                            pf[d][:, 512:1024], bias_hi[d][:], onehot2[:],
                            start=True, stop=False, skip_group_check=True))
                        for gi in range(4):
                            thunks.append(lambda d=d, gi=gi, x0=x0: nc.tensor.matmul(
                                pf[d][:, gi * REG:(gi + 1) * REG],
                                wiht[d][:, gi * 128:(gi + 1) * 128],
                                x[:, x0:x0 + W * BLOC],
                                start=False, stop=False, skip_group_check=True))
                    return thunks

                def alloc_pf():
                    return {d: lpsp.tile([128, 4 * REG], DT.float32,
                                         tag=f"pf{d}", name=f"pf{d}") for d in "fb"}

                pf = alloc_pf()
                for th in window_mm_thunks(0, pf):
                    th()
                for w in range(NW):
                    pf_next = None
                    next_thunks = []
                    for s in range(W):
                        if s == 1 and w + 1 < NW:
                            pf_next = alloc_pf()
                            next_thunks = window_mm_thunks(w + 1, pf_next)
                        for _ in range(2):
                            if next_thunks:
                                next_thunks.pop(0)()

                        st = {}
                        for d in "fb":
                            if d == "f":
                                t = w * W + s
                                slot = s
                                tprev_col = (t - 1) * BLOC
                                first = (t == 0)
                            else:
                                t = L - 1 - (w * W + s)
                                slot = W - 1 - s
                                tprev_col = (t + 1) * BLOC
                                first = (t == L - 1)
                            st[d] = (t, slot, tprev_col, first)
                            if not first:
                                for gi in range(4):
                                    nc.tensor.matmul(
                                        pf[d][:, gi * REG + slot * 32: gi * REG + (slot + 1) * 32],
                                        whht[d][:, gi * 128:(gi + 1) * 128],
                                        Hdir[d][:, tprev_col:tprev_col + BLOC],
                                        start=False, stop=True, skip_group_check=True)
                        Sgs = {}
                        for d in "fb":
                            t, slot, tprev_col, first = st[d]
                            Sg = lsb.tile([128, 128], DT.bfloat16, tag=f"S{d}", name=f"S{d}")
                            Sgs[d] = Sg
                            pf3 = pf[d][:].rearrange("p (g n) -> p g n", g=4)
                            Sc.activation(
                                Sg[:].rearrange("p (g n) -> p g n", g=4),
                                pf3[:, :, slot * 32:(slot + 1) * 32],
                                AF.Sigmoid)
                        # f's full pointwise chain first: the f-chain is the
                        # critical path; b trails it with slack and fills gaps
                        for d in "fb":
                            t, slot, tprev_col, first = st[d]
                            Sg = Sgs[d]
                            if first:
                                Vv.affine_mul_reduce(
                                    out=cst[d][:], accum_out=jacc[d][:],
                                    in0=Sg[:, 96:128], in1=Sg[:, 0:32],
                                    scale=2.0, bias=-1.0)
                            else:
                                Vv.tensor_tensor(out=tmp1[d][:], in0=Sg[:, 32:64],
                                                 in1=cst[d][:], op=ALU.mult)
                                Vv.affine_mul_reduce(
                                    out=tmp2[d][:], accum_out=jacc[d][:],
                                    in0=Sg[:, 96:128], in1=Sg[:, 0:32],
                                    scale=2.0, bias=-1.0)
                                Vv.tensor_tensor(out=cst[d][:], in0=tmp1[d][:],
                                                 in1=tmp2[d][:], op=ALU.add)
                            Sc.activation(tct[d][:], cst[d][:], AF.Tanh)
                            Vv.tensor_tensor(out=Hdir[d][:, t * BLOC:(t + 1) * BLOC],
                                             in0=Sg[:, 64:96], in1=tct[d][:],
                                             op=ALU.mult)

                    while next_thunks:
                        next_thunks.pop(0)()
                    if pf_next is not None:
                        pf = pf_next

            # ---------------- tail: emissions + numerator + fused CRF -------
            with tc.tile_pool(name="em_ps", bufs=2, space="PSUM") as epsp, \
                 tc.tile_pool(name="crf_ps", bufs=2, space="PSUM") as kpsp, \
                 tc.tile_pool(name="fin_ps", bufs=1, space="PSUM") as fpsp:

                def emit_chunk(k):
                    cs = k * 512
                    emps = epsp.tile([112, 512], DT.float32, tag="emps", name="emps")
                    nc.tensor.matmul(emps[:], wot112["f"][:], Hf[:, cs:cs + 512],
                                     start=True, stop=False, skip_group_check=True)
                    nc.tensor.matmul(emps[:], wot112["b"][:], Hb[:, cs:cs + 512],
                                     start=False, stop=True, skip_group_check=True)
                    if k <= 15:
                        # top half: EM blocks 16k..16k+15 in natural order
                        Sc.activation(EMpair[0:48, cs:cs + 512], emps[0:48, :],
                                      AF.Exp, bias=exbias112[0:48, :])
                    else:
                        # bottom half: block t -> col-block 512-t (reversed)
                        lo = EMpair[64:112, :]
                        rev = bass.AP(tensor=lo.tensor,
                                      offset=lo.offset + (512 - 16 * k) * 32,
                                      ap=[list(lo.ap[0]), [-32, 16], [1, 32]])
                        Sc.activation(
                            rev,
                            emps[64:112, :].rearrange("p (b n) -> p b n", n=32),
                            AF.Exp, bias=exbias112[64:112, :])
                    # numerator A part: gather emissions at gold tags
                    Vv.affine_mul_reduce(
                        out=junkA[:], accum_out=accA[:, k:k + 1],
                        in0=emps[0:48, :], in1=ohm_all[:, cs:cs + 512],
                        scale=1.0, bias=0.0)

                emit_chunk(0)
                emit_chunk(31)
                # alpha init: ea0 top = EM[0] * exp(start_trans); bottom = 0
                Vv.tensor_scalar(ea[0][0:48, :], EMpair[0:48, 0:32], estart[:],
                                 None, ALU.mult)
                pa = kpsp.tile([112, BLOC], DT.float32, tag="pa", name="pa")
                nc.tensor.matmul(pa[:], eend112[:], lsrow[0:1, 511 * 32:512 * 32],
                                 start=True, stop=False, skip_group_check=True)
                nc.tensor.matmul(pa[:], bd112[:], ea[0][:], start=False, stop=True,
                                 skip_group_check=True)
                cur = 0
                pa_prev = pa
                for j in range(16):
                    if j < 15:
                        emit_chunk(j + 1)
                        emit_chunk(30 - j)
                    for s in range(max(1, 16 * j), 16 * j + 16):
                        cur ^= 1
                        Vv.tensor_tensor(out=ea[cur][:], in0=pa_prev[:],
                                         in1=EMpair[:, s * 32:(s + 1) * 32],
                                         op=ALU.mult)
                        pa = kpsp.tile([112, BLOC], DT.float32, tag="pa", name="pa")
                        nc.tensor.matmul(pa[:], eend112[:],
                                         lsrow[0:1, (511 - s) * 32:(512 - s) * 32],
                                         start=True, stop=False, skip_group_check=True)
                        nc.tensor.matmul(pa[:], bd112[:], ea[cur][:],
                                         start=False, stop=True, skip_group_check=True)
                        pa_prev = pa
                acur = cur  # ea[acur] top holds alpha_255
                for s in (256,):
                    cur ^= 1
                    Vv.tensor_tensor(out=ea[cur][64:112, :], in0=pa_prev[64:112, :],
                                     in1=EMpair[64:112, s * 32:(s + 1) * 32],
                                     op=ALU.mult)
                    pa = kpsp.tile([112, BLOC], DT.float32, tag="pa", name="pa")
                    nc.tensor.matmul(pa[:], eend112[:],
                                     lsrow[0:1, (511 - s) * 32:(512 - s) * 32],
                                     start=True, stop=False, skip_group_check=True)
                    nc.tensor.matmul(pa[:], bd112[:], ea[cur][:],
                                     start=False, stop=True, skip_group_check=True)
                    pa_prev = pa
                # combine: denom_seq = log(sum_j alpha_j * G_j)
                S.dma_start(out=dcopy[64:112, :], in_=ea[acur][0:48, :])
                Vv.tensor_tensor(out=dott112[64:112, :], in0=pa_prev[64:112, :],
                                 in1=dcopy[64:112, :], op=ALU.mult)
                S.dma_start(out=dott_lo[:], in_=dott112[64:112, :])
                fint = fpsp.tile([1, 64], DT.float32, name="fint")
                nc.tensor.matmul(fint[:, 0:BLOC], ones48col[:],
                                 dott_lo[:], start=True, stop=True,
                                 skip_group_check=True)
                Sc.activation(logrow[:], fint[:, 0:BLOC], AF.Ln)
                Vv.tensor_reduce(dsum[:], logrow[:], mybir.AxisListType.X, ALU.add)

                # numerator A total
                Vv.tensor_reduce(accA_red[:], accA[:], mybir.AxisListType.X, ALU.add)
                nc.tensor.matmul(fint[:, 34:35], accA_red[:], ones48col[:], start=True,
                                 stop=True, skip_group_check=True)
                # out[0] = A partial sum ; out[1] = log-dot partial sum
                Vv.tensor_copy(out_sb[:, 0:1], fint[:, 34:35])
                Vv.tensor_copy(out_sb[:, 1:2], dsum[:])
            S.dma_start(out=g_out[:], in_=out_sb[:])

    return nc


# --------------------------------------------------------------------------
# self-contained entry point: kernel(**inputs) -> scalar loss (numpy)
# --------------------------------------------------------------------------

_CACHED = {}


def _get_nc():
    if "nc" not in _CACHED:
        nc = build(L=512, BLOC=32, W=8, V=32000)
        if not nc.is_finalized():
            nc.finalize()
        _CACHED["nc"] = nc
    return _CACHED["nc"]


def kernel(**inputs):
    from concourse.bass_utils import run_bass_kernel_spmd

    B = 256
    BLOC = B // 8
    p = prep_params(inputs)
    in_maps = []
    words = np.asarray(inputs["words"])
    tags = np.asarray(inputs["tags"])
    mask = np.asarray(inputs["mask"])
    for core in range(8):
        sl = slice(core * BLOC, (core + 1) * BLOC)
        d = prep_shard(words[sl], tags[sl], mask[sl])
        d.update(p)
        in_maps.append(d)
    nc = _get_nc()
    res = run_bass_kernel_spmd(nc, in_maps, list(range(8)))
    return np.float32(combine(res.results, inputs))


def combine(results, inputs):
    B = 256
    tot_A = sum(float(r["out"][0, 0]) for r in results)
    tot_ds = sum(float(r["out"][0, 1]) for r in results)
    num_host, den_extra = host_extra(
        np.asarray(inputs["tags"]), np.asarray(inputs["mask"]),
        inputs["trans"], inputs["start_trans"], inputs["end_trans"],
        inputs["bout"])
    return ((tot_ds + den_extra) - (tot_A + num_host)) / B
